# revision 46
# baseline (speedup 1.0000x reference)
"""Trainium2 Bass kernel for nn_BasePriorNetwork (4-layer dense transformer).

Sharding: data-parallel over batch (B=8) across 8 NeuronCores; weights
replicated. Activations feature-major ([feat, token]) on-chip. All GEMMs in
bf16 (fp32 PSUM accumulation); LN stats and residual stream in fp32.

Key structure vs the original port:
 - All weights are host-prepacked into exact SBUF tile images (gains folded
   in) and streamed with a handful of large HWDGE DMAs per layer.
 - The relative-position bias is layer-invariant; exp(bias) (with the causal
   mask folded in as zeros) is loaded once and applied multiplicatively to
   exp(scores).
 - The softmax denominator comes free from a ones-column appended to V.
 - Rotary embedding is computed with one +-1-permutation matmul and three
   vector ops per 128-row tile (two heads at a time for Q).
"""
import sys, math, os
sys.path.insert(0, '/opt/trn_rl_repo')
import numpy as np
import ml_dtypes

import concourse.bass as bass
import concourse.bacc as bacc
import concourse.tile as tile
from concourse import mybir, bass_isa

f32 = mybir.dt.float32
f32r = mybir.dt.float32r
bf16 = mybir.dt.bfloat16
fp16 = mybir.dt.float16
AF = mybir.ActivationFunctionType
ALU = mybir.AluOpType

B, N, D = 8, 515, 1024
H, DH, L = 8, 64, 4
FF = 4 * D
ROT = 32
NB, MAXD = 32, 128
EPS = 1e-5
SCALE = 16.0

NP = 520                      # padded tokens / keys / queries
QT = 260
QTS = [(0, QT), (QT, QT)]
KCH = [(0, 128), (128, 128), (256, 128), (384, 128), (512, 8)]
TCH = KCH
NMT = D // 128                # 8 feature tiles of x

# per-head dim permutation: even rot dims -> 0:16, odd -> 16:32, pass -> 32:64
PERM2 = list(range(0, ROT, 2)) + list(range(1, ROT, 2)) + list(range(ROT, DH))

bfd = ml_dtypes.bfloat16
hfd = np.float16


def _np_expbias(emb):
    """EB[h, key, query] = exp(rel-pos bias) with causal mask / padding folded
    in as exact zeros. keys: 0..514 tokens, 515..518 pad, 519 null."""
    q_pos = np.arange(N)
    k_pos = np.arange(N + 1)
    rel = k_pos[None, :] - q_pos[:, None]
    nn = np.maximum(-rel, 0)
    max_exact = NB // 2
    is_small = nn < max_exact
    nf = np.maximum(nn, 1).astype(np.float32)
    val_large = max_exact + (
        np.log(nf / np.float32(max_exact)).astype(np.float32)
        / np.float32(math.log(MAXD / max_exact)) * np.float32(NB - max_exact)
    ).astype(np.int32)
    val_large = np.minimum(val_large, NB - 1)
    bucket = np.where(is_small, nn, val_large)           # [n, n+1]
    bias = emb[bucket]                                    # [n, n+1, H]
    bias = np.transpose(bias, (2, 0, 1)).astype(np.float32)  # [H, q, col]

    out = np.zeros((H, NP, NP), np.float32)               # [h, key, query]
    # token keys j at col j+1; allowed iff j <= q
    tok = np.exp(np.transpose(bias[:, :, 1:], (0, 2, 1)))  # [H, key, query]
    jj = np.arange(N)[:, None]
    qq = np.arange(N)[None, :]
    tok = np.where(jj > qq, np.float32(0.0), tok)
    out[:, :N, :N] = tok
    out[:, NP - 1, :N] = np.exp(bias[:, :, 0])            # null key col
    out[:, NP - 1, N:] = 1.0                              # pad queries -> null only
    return out


def _host_prep(inputs):
    f = np.float32
    x = np.asarray(inputs['x'], f)
    ag = np.asarray(inputs['attn_norm_g'], f)
    og = np.asarray(inputs['out_norm_g'], f)
    fg = np.asarray(inputs['ff_norm_g'], f)
    lg = np.asarray(inputs['final_norm_g'], f)
    Wq = np.asarray(inputs['Wq'], f)
    Wkv = np.asarray(inputs['Wkv'], f)
    bkv = np.asarray(inputs['bkv'], f)
    null_kv = np.asarray(inputs['null_kv'], f)
    Wo = np.asarray(inputs['Wo'], f)
    Wff1 = np.asarray(inputs['Wff1'], f)
    Wff2 = np.asarray(inputs['Wff2'], f)
    relpos_emb = np.asarray(inputs['relpos_emb'], f)
    Wproj = np.asarray(inputs['Wproj'], f)

    d = {}
    # x feature-major: xt[p, c*NP+n] = x[b, n, c*128+p]
    xT = np.zeros((B, 128, NMT * NP), f)
    for c in range(NMT):
        xT[:, :, c * NP:c * NP + N] = np.transpose(x[:, :, c * 128:(c + 1) * 128], (0, 2, 1))
    d['xT'] = xT

    # ---- wq: [L, 128, 4*8*128] bf16, gains folded, PERM2, 2 heads per tile
    Wq_g = ag[:, :, None] * Wq                          # [L, D, 512]
    A = Wq_g.reshape(L, 8, 128, H, DH)[:, :, :, :, PERM2]   # [L,c,p,h,d]
    Aq = A.reshape(L, 8, 128, 4, 128)                   # [L,c,p,t,hh*64+d]
    d['wq'] = np.ascontiguousarray(
        Aq.transpose(0, 2, 3, 1, 4).reshape(L, 128, 4 * 8 * 128)).astype(hfd)

    # ---- smallw: [L, 128, 1024] = wk(512) | wv(512)
    smallw = np.zeros((L, 128, 1024), f)
    Wk_g = (ag[:, :, None] * Wkv[:, :, :DH])[:, :, PERM2]   # [L, D, 64]
    smallw[:, :, 0:512] = Wk_g.reshape(L, 8, 128, DH).transpose(0, 2, 1, 3).reshape(L, 128, 512)
    Wv_g = ag[:, :, None] * Wkv[:, :, DH:]
    smallw[:, :, 512:1024] = Wv_g.reshape(L, 8, 128, DH).transpose(0, 2, 1, 3).reshape(L, 128, 512)
    d['smallw'] = np.ascontiguousarray(smallw).astype(hfd)
    # wo: lhsT block (cc, mt): wo[p, cc*1024+mt*128+m] =
    #     Wo[(2cc + p//64)*64 + p%64, mt*128+m]  (= Wo[cc*128+p, mt*128+m])
    Wo_r = Wo.reshape(L, 4, 128, D)                     # [L, cc, p(row in tile), D]
    d['wo'] = np.ascontiguousarray(
        Wo_r.transpose(0, 2, 1, 3).reshape(L, 128, 4 * D)).astype(hfd)

    # ---- wff1: [L, 8, 128, 8*1024]; stage s=2q+hh has tiles
    #      [g(q8+4hh), a(q8+4hh), g(+1), a(+1), g(+2), a(+2), g(+3), a(+3)]
    W1_g = fg[:, :, None] * Wff1                        # [L, D, 2FF]
    W1a = W1_g[:, :, :FF].reshape(L, 8, 128, 32, 128)   # [L,c,p,mt,m]
    W1g = W1_g[:, :, FF:].reshape(L, 8, 128, 32, 128)
    wff1 = np.zeros((L, 8, 128, 8 * 1024), f)
    for s in range(8):
        for j in range(4):
            mt = (s // 2) * 8 + (s % 2) * 4 + j
            ga = W1g[:, :, :, mt, :].transpose(0, 2, 1, 3).reshape(L, 128, 1024)
            aa = W1a[:, :, :, mt, :].transpose(0, 2, 1, 3).reshape(L, 128, 1024)
            wff1[:, s, :, (2 * j) * 1024:(2 * j + 1) * 1024] = ga
            wff1[:, s, :, (2 * j + 1) * 1024:(2 * j + 2) * 1024] = aa
    d['wff1'] = np.ascontiguousarray(wff1).astype(hfd)

    # ---- wff2: [L, 8, 128, 4096]; stage u = q*2+hh covers out m-tiles
    #      (hh*4..hh*4+4), contraction = quarter q's 8 fc chunks
    W2r = Wff2.reshape(L, 4, 8, 128, D)                 # [L, q, fc, p, D]
    wff2 = np.zeros((L, 8, 128, 4096), f)
    for q in range(4):
        for hh in range(2):
            for mo in range(4):
                mt = hh * 4 + mo
                blk = W2r[:, q, :, :, mt * 128:(mt + 1) * 128]  # [L, fc, p, 128]
                wff2[:, q * 2 + hh, :, mo * 1024:(mo + 1) * 1024] = \
                    blk.transpose(0, 2, 1, 3).reshape(L, 128, 1024)
    d['wff2'] = np.ascontiguousarray(wff2).astype(hfd)

    # ---- wproj: [128, 2*8*512] bf16 (final gain folded)
    Wp_g = lg[:, None] * Wproj
    Wp = Wp_g.reshape(8, 128, 2, 512).transpose(1, 2, 0, 3).reshape(128, 2 * 8 * 512)
    d['wproj'] = np.ascontiguousarray(Wp).astype(hfd)

    # ---- expbias: [128, 5*8*520] bf16: eb[p, (c*8+h)*520+n] = EB[h, c*128+p, n]
    EB = _np_expbias(relpos_emb)                        # [H, key, query]
    ebt = np.zeros((128, 5 * 8 * NP), f)
    for c in range(5):
        kw = min(128, NP - c * 128)
        for h in range(H):
            ebt[:kw, (c * 8 + h) * NP:(c * 8 + h + 1) * NP] = EB[h, c * 128:c * 128 + kw, :]
    d['ebias'] = np.ascontiguousarray(ebt).astype(bfd)

    # ---- rotary helpers [128, NP] bf16 and P swap matrix [128,128] bf16
    inv_freq = (1.0 / (10000.0 ** (np.arange(0, ROT, 2, dtype=f) / ROT))).astype(f)
    fr = np.arange(NP, dtype=f)[None, :] * inv_freq[:, None]   # [16, NP]
    cosf, sinf = np.cos(fr), np.sin(fr)
    hcc = np.ones((128, NP), f)
    hss = np.zeros((128, NP), f)
    for base in (0, 64):
        hcc[base:base + 16] = cosf
        hcc[base + 16:base + 32] = cosf
        hss[base:base + 16] = sinf
        hss[base + 16:base + 32] = sinf
    d['hcc'] = hcc.astype(hfd)
    d['hss'] = hss.astype(hfd)
    P = np.zeros((128, 128), f)                         # qswap = P.T @ q
    for base in (0, 64):
        for i in range(16):
            P[base + 16 + i, base + i] = -1.0           # out[0:16] = -q[16:32]
            P[base + i, base + 16 + i] = 1.0            # out[16:32] = q[0:16]
    d['pswap'] = P.astype(hfd)

    # ---- small consts
    d['nullk'] = np.ascontiguousarray(null_kv[:, 0, PERM2][:, :, None]).astype(hfd)  # [L,64,1]
    d['nullv'] = np.ascontiguousarray(null_kv[:, 1, :][:, None, :]).astype(bfd)      # [L,1,64]
    d['bk'] = np.ascontiguousarray(bkv[:, :DH][:, PERM2].reshape(L, 1, DH)).astype(hfd)
    d['bv'] = np.ascontiguousarray(bkv[:, DH:].reshape(L, 1, DH)).astype(hfd)
    d['ogain'] = np.ascontiguousarray(og.reshape(L, 1, D))  # [L,1,1024] f32
    d['ones128c_f'] = np.ones((128, 1), f)
    d['ones128c_b'] = np.ones((128, 1), f).astype(hfd)
    d['ones1x128'] = np.ones((1, 128), f)
    d['ones1xNP_b'] = np.ones((1, NP), f).astype(hfd)
    d['ones1x64'] = np.ones((1, 64), f)
    d['blk2'] = np.ascontiguousarray(
        np.kron(np.eye(2, dtype=f), np.ones((64, 1), f))).astype(hfd)  # [128,2]
    bx = np.kron(np.eye(2, dtype=f), np.ones((1, 64), f))              # [2,128]
    d['blkx16'] = np.ascontiguousarray(bx * SCALE)       # [2,128] f32 (16x for q)
    d['ones64c_b'] = np.ones((64, 1), f).astype(hfd)
    return d


def _build():
    nc = bacc.Bacc("TRN2", target_bir_lowering=False, debug=False, num_devices=8)

    def P(name, shape, dt=f32):
        return nc.declare_dram_parameter(name, list(shape), dt, isOutput=False)

    xT_d = P('xT', [128, NMT * NP])
    wq_d = P('wq', [L, 128, 4096], fp16)
    wo_d = P('wo', [L, 128, 4096], fp16)
    smallw_d = P('smallw', [L, 128, 1024], fp16)
    wff1_d = P('wff1', [L, 8, 128, 8192], fp16)
    wff2_d = P('wff2', [L, 8, 128, 4096], fp16)
    wproj_d = P('wproj', [128, 8192], fp16)
    ebias_d = P('ebias', [128, 40 * NP], bf16)
    hcc_d = P('hcc', [128, NP], fp16)
    hss_d = P('hss', [128, NP], fp16)
    pswap_d = P('pswap', [128, 128], fp16)
    nullk_d = P('nullk', [L, DH, 1], fp16)
    nullv_d = P('nullv', [L, 1, DH], bf16)
    bk_d = P('bk', [L, 1, DH], fp16)
    bv_d = P('bv', [L, 1, DH], fp16)
    ogain_d = P('ogain', [L, 1, D])
    ones128c_f_d = P('ones128c_f', [128, 1])
    ones128c_b_d = P('ones128c_b', [128, 1], fp16)
    ones1x128_d = P('ones1x128', [1, 128])
    ones1xNP_b_d = P('ones1xNP_b', [1, NP], fp16)
    ones1x64_d = P('ones1x64', [1, 64])
    blk2_d = P('blk2', [128, 2], fp16)
    blkx16_d = P('blkx16', [2, 128])
    ones64c_b_d = P('ones64c_b', [64, 1], fp16)
    out_d = nc.declare_dram_parameter('out', [N, D], f32, isOutput=True)

    R = f32r

    with nc.allow_low_precision("bf16 data path; fp32 accumulation in PSUM"), \
         tile.TileContext(nc) as tc:
        with tc.tile_pool(name="const", bufs=1) as cpool, \
             tc.tile_pool(name="res", bufs=1) as rpool, \
             tc.tile_pool(name="wq", bufs=2) as wqp, \
             tc.tile_pool(name="wsm", bufs=2) as wsp, \
             tc.tile_pool(name="wf1", bufs=2) as wf1p, \
             tc.tile_pool(name="wf2", bufs=2) as wf2p, \
             tc.tile_pool(name="scr", bufs=2) as scrp, \
             tc.tile_pool(name="att", bufs=3) as attp, \
             tc.tile_pool(name="vec", bufs=1) as vecp, \
             tc.tile_pool(name="fin", bufs=1) as finp, \
             tc.tile_pool(name="ps", bufs=4, space="PSUM") as ps, \
             tc.tile_pool(name="psacc", bufs=2, space="PSUM") as psacc, \
             tc.tile_pool(name="psbc", bufs=2, space="PSUM") as psbc:

            # ---------- constants (one-time loads) ----------
            hcc_t = cpool.tile([128, NP], fp16, tag="hcc")
            nc.sync.dma_start(hcc_t[:], hcc_d[:])
            hss_t = cpool.tile([128, NP], fp16, tag="hss")
            nc.sync.dma_start(hss_t[:], hss_d[:])
            pswap_t = cpool.tile([128, 128], fp16, tag="pswap")
            nc.sync.dma_start(pswap_t[:], pswap_d[:])
            ones128c_f = cpool.tile([128, 1], R, tag="o128f")
            nc.sync.dma_start(ones128c_f[:], ones128c_f_d[:].bitcast(R))
            ones128c_b = cpool.tile([128, 1], fp16, tag="o128b")
            nc.sync.dma_start(ones128c_b[:], ones128c_b_d[:])
            ones1x128_t = cpool.tile([1, 128], R, tag="o1x128")
            nc.sync.dma_start(ones1x128_t[:], ones1x128_d[:].bitcast(R))
            ones1xNP_b = cpool.tile([1, NP], fp16, tag="o1xNPb")
            nc.sync.dma_start(ones1xNP_b[:], ones1xNP_b_d[:])
            ones1x64_t = cpool.tile([1, 64], R, tag="o1x64")
            nc.sync.dma_start(ones1x64_t[:], ones1x64_d[:].bitcast(R))
            blk2_t = cpool.tile([128, 2], fp16, tag="blk2")
            nc.sync.dma_start(blk2_t[:], blk2_d[:])
            blkx16_t = cpool.tile([2, 128], R, tag="blkx16")
            nc.sync.dma_start(blkx16_t[:], blkx16_d[:].bitcast(R))
            ones64c_b = cpool.tile([64, 1], fp16, tag="o64b")
            nc.sync.dma_start(ones64c_b[:], ones64c_b_d[:])
            epsc = cpool.tile([128, 1], f32, tag="epsc")
            nc.gpsimd.memset(epsc[:], EPS)
            eps12 = cpool.tile([128, 1], f32, tag="eps12")
            nc.gpsimd.memset(eps12[:], 1e-12)

            # ---------- persistent activations ----------
            xt = rpool.tile([128, NMT * NP], R, tag="x")
            nc.sync.dma_start(xt[:], xT_d[:].bitcast(R))
            xn = rpool.tile([128, NMT * NP], fp16, tag="xn")
            sff = rpool.tile([128, NMT * NP], fp16, tag="sff")
            qhat = rpool.tile([128, 4 * NP], fp16, tag="qhat")
            oT = rpool.tile([128, 4 * NP], fp16, tag="oT")
            kdup = rpool.tile([128, NP], fp16, tag="kdup")
            vaug = rpool.tile([128, 5 * 65], bf16, tag="vaug")
            for c in range(5):
                nc.gpsimd.memset(vaug[:, c * 65 + 64:c * 65 + 65], 1.0)

            # per-layer weight stages (double buffered via pool bufs=2)
            def load_wq(l):
                t = wqp.tile([128, 4096], fp16, tag="wq")
                nc.sync.dma_start(t[:], wq_d[l])
                return t

            def load_smallw(l):
                t = wsp.tile([128, 1024], fp16, tag="smallw")
                nc.sync.dma_start(t[:], smallw_d[l])
                return t

            def load_wo(l):
                t = wqp.tile([128, 4096], fp16, tag="wq")
                nc.sync.dma_start(t[:], wo_d[l])
                return t

            def load_og(l):
                t = wsp.tile([1, D], R, tag="og")
                nc.sync.dma_start(t[:], ogain_d[l].bitcast(R))
                return t

            def ln_stats(src_tile, qo, qw, dt_src):
                """means/rstd for one query tile; returns (m_v, r_v) [1,qw] f32/f32r"""
                s1p = ps.tile([1, QT], f32, tag="mm")
                s2p = ps.tile([1, QT], f32, tag="mm")
                ones = ones128c_f if dt_src == R else ones128c_b
                sqtag = "sqR" if dt_src == R else "sqB"
                for mt in range(NMT):
                    seg = src_tile[:, mt * NP + qo:mt * NP + qo + qw]
                    sq = scrp.tile([128, QT], dt_src, tag=sqtag)
                    nc.scalar.activation(sq[:], seg, AF.Square)
                    nc.tensor.matmul(s1p[:], ones[:], seg,
                                     start=(mt == 0), stop=(mt == NMT - 1))
                    nc.tensor.matmul(s2p[:], ones[:], sq[:],
                                     start=(mt == 0), stop=(mt == NMT - 1))
                m_v = vecp.tile([1, QT], R, tag="m")
                nc.scalar.activation(m_v[:], s1p[:], AF.Copy, scale=1.0 / D)
                q2_v = vecp.tile([1, QT], f32, tag="q2")
                nc.scalar.activation(q2_v[:], s2p[:], AF.Copy, scale=1.0 / D)
                msq_v = vecp.tile([1, QT], f32, tag="msq")
                nc.scalar.activation(msq_v[:], m_v[:], AF.Square)
                v_v = vecp.tile([1, QT], f32, tag="v")
                nc.vector.tensor_sub(v_v[:], q2_v[:], msq_v[:])
                s_v = vecp.tile([1, QT], f32, tag="s")
                nc.scalar.activation(s_v[:], v_v[:], AF.Sqrt, bias=epsc[0:1, :])
                r_v = vecp.tile([1, QT], R, tag="r")
                nc.vector.reciprocal(r_v[:], s_v[:])
                return m_v, r_v

            def ln_to_xn(src_tile, dt_src):
                """xn = (src - m) * r  (gain folded into consumer weights).
                Applies over all 8 feature tiles in one batched op per step."""
                for (qo, qw) in QTS:
                    m_v, r_v = ln_stats(src_tile, qo, qw, dt_src)
                    mr_v = vecp.tile([1, QT], R, tag="mr")
                    nc.vector.tensor_mul(mr_v[:], m_v[:], r_v[:])
                    rb = psbc.tile([128, QT], f32, tag="bc")
                    nc.tensor.matmul(rb[:], ones1x128_t[:], r_v[:], start=True, stop=True)
                    mrb = psbc.tile([128, QT], f32, tag="bc")
                    nc.tensor.matmul(mrb[:], ones1x128_t[:], mr_v[:], start=True, stop=True)
                    src_v = src_tile[:].rearrange("p (c n) -> p c n", c=NMT)[:, :, qo:qo + qw]
                    xn_v = xn[:].rearrange("p (c n) -> p c n", c=NMT)[:, :, qo:qo + qw]
                    nc.vector.tensor_mul(xn_v, src_v,
                                         rb[:, None, :].broadcast_to([128, NMT, qw]))
                    nc.vector.tensor_sub(xn_v, xn_v,
                                         mrb[:, None, :].broadcast_to([128, NMT, qw]))

            def ffn_up_quarter(l, q):
                for hh in range(2):
                    s = q * 2 + hh
                    for jp in range(2):
                        w1 = wf1p.tile([128, 4096], fp16, tag="wff1")
                        nc.sync.dma_start(w1[:], wff1_d[l, s, :, jp * 4096:(jp + 1) * 4096])
                        for (j2, (qo, qw)) in [(a, b) for a in range(2) for b in QTS]:
                            fc = hh * 4 + jp * 2 + j2
                            pg = ps.tile([128, QT], f32, tag="mm")
                            for c in range(8):
                                nc.tensor.matmul(
                                    pg[:], w1[:, (2 * j2) * 1024 + c * 128:(2 * j2) * 1024 + (c + 1) * 128],
                                    xn[:, c * NP + qo:c * NP + qo + qw],
                                    start=(c == 0), stop=(c == 7))
                            sg = attp.tile([128, QT], fp16, tag="sg")
                            if os.environ.get('KSIM'):
                                sig = attp.tile([128, QT], f32, tag="sig")
                                nc.scalar.activation(sig[:], pg[:], AF.Sigmoid)
                                nc.vector.tensor_mul(sg[:], pg[:], sig[:])
                            else:
                                nc.scalar.activation(sg[:], pg[:], AF.Silu)
                            pa = ps.tile([128, QT], f32, tag="mm")
                            for c in range(8):
                                nc.tensor.matmul(
                                    pa[:], w1[:, (2 * j2 + 1) * 1024 + c * 128:(2 * j2 + 1) * 1024 + (c + 1) * 128],
                                    xn[:, c * NP + qo:c * NP + qo + qw],
                                    start=(c == 0), stop=(c == 7))
                            pa_s = attp.tile([128, QT], fp16, tag="pas")
                            nc.scalar.copy(pa_s[:], pa[:])
                            nc.vector.tensor_mul(sff[:, fc * NP + qo:fc * NP + qo + qw],
                                                 pa_s[:], sg[:])

            def ffn_down_quarter(l, q):
                for hh in range(2):
                    u = q * 2 + hh
                    w2 = wf2p.tile([128, 4096], fp16, tag="wff2")
                    nc.sync.dma_start(w2[:], wff2_d[l, u])
                    for (mo, (qo, qw)) in [(a, b) for a in range(4) for b in QTS]:
                        mt = hh * 4 + mo
                        pl = ps.tile([128, QT], f32, tag="mm")
                        for fc in range(8):
                            nc.tensor.matmul(
                                pl[:], w2[:, mo * 1024 + fc * 128:mo * 1024 + (fc + 1) * 128],
                                sff[:, fc * NP + qo:fc * NP + qo + qw],
                                start=(fc == 0), stop=(fc == 7))
                        xcols = xt[:, mt * NP + qo:mt * NP + qo + qw]
                        nc.vector.tensor_add(xcols, xcols, pl[:])

            # ================= layers =================
            wq_t = load_wq(0)
            sw_t = load_smallw(0)
            og_t = load_og(0)
            # ebias is first needed mid-way through layer 0's attention; issue
            # its (large) load after the critical startup tensors.
            ebias_t = cpool.tile([128, 40 * NP], bf16, tag="ebias")
            nc.sync.dma_start(ebias_t[:], ebias_d[:])
            for l in range(L):
                ln_to_xn(xt, R)

                # ---- Q: 4 tiles x 2 heads ----
                for t in range(4):
                    for qi, (qo, qw) in enumerate(QTS):
                        pq = ps.tile([128, QT], f32, tag="mm")
                        for c in range(8):
                            nc.tensor.matmul(pq[:], wq_t[:, (t * 8 + c) * 128:(t * 8 + c + 1) * 128],
                                             xn[:, c * NP + qo:c * NP + qo + qw],
                                             start=(c == 0), stop=(c == 7))
                        pqs = scrp.tile([128, QT], fp16, tag="pqs")
                        nc.scalar.copy(pqs[:], pq[:])
                        t1 = scrp.tile([128, QT], fp16, tag="t1b")
                        nc.vector.tensor_mul(t1[:], pqs[:], hcc_t[:, qo:qo + qw])
                        qsw = ps.tile([128, QT], f32, tag="mm")
                        nc.tensor.matmul(qsw[:], pswap_t[:], pqs[:], start=True, stop=True)
                        t2 = scrp.tile([128, QT], fp16, tag="t2b")
                        nc.vector.tensor_mul(t2[:], qsw[:], hss_t[:, qo:qo + qw])
                        qr = scrp.tile([128, QT], fp16, tag="qr")
                        nc.vector.tensor_add(qr[:], t1[:], t2[:])
                        sq = scrp.tile([128, QT], fp16, tag="qsq")
                        nc.scalar.activation(sq[:], qr[:], AF.Square)
                        ssq = ps.tile([2, QT], f32, tag="mm")
                        nc.tensor.matmul(ssq[:], blk2_t[:], sq[:], start=True, stop=True)
                        sh = vecp.tile([2, QT], f32, tag="sh")
                        nc.scalar.activation(sh[:], ssq[:], AF.Sqrt, bias=eps12[0:2, :])
                        rh = vecp.tile([2, QT], R, tag="rh")
                        nc.vector.reciprocal(rh[:], sh[:])
                        rbq = psbc.tile([128, QT], f32, tag="bc")
                        nc.tensor.matmul(rbq[:], blkx16_t[:], rh[:], start=True, stop=True)
                        nc.vector.tensor_mul(qhat[:, t * NP + qo:t * NP + qo + qw],
                                             qr[:], rbq[:])

                # ---- K ----
                bkr = scrp.tile([1, DH], fp16, tag="bkr")
                nc.sync.dma_start(bkr[:], bk_d[l])
                ks = scrp.tile([64, NP], fp16, tag="ks")
                for (qo, qw) in QTS:
                    pk = ps.tile([64, QT], f32, tag="mm")
                    for c in range(8):
                        nc.tensor.matmul(pk[:], sw_t[:, c * 64:(c + 1) * 64],
                                         xn[:, c * NP + qo:c * NP + qo + qw],
                                         start=(c == 0), stop=False)
                    nc.tensor.matmul(pk[:], bkr[:], ones1xNP_b[:, qo:qo + qw],
                                     start=False, stop=True)
                    pks = scrp.tile([64, QT], fp16, tag="pks")
                    nc.scalar.copy(pks[:], pk[:])
                    t1 = scrp.tile([64, QT], fp16, tag="t1b")
                    nc.vector.tensor_mul(t1[:], pks[:], hcc_t[0:64, qo:qo + qw])
                    ksw = ps.tile([64, QT], f32, tag="mm")
                    nc.tensor.matmul(ksw[:], pswap_t[0:64, 0:64], pks[:], start=True, stop=True)
                    t2 = scrp.tile([64, QT], fp16, tag="t2b")
                    nc.vector.tensor_mul(t2[:], ksw[:], hss_t[0:64, qo:qo + qw])
                    nc.vector.tensor_add(ks[:, qo:qo + qw], t1[:], t2[:])
                nc.sync.dma_start(ks[:, NP - 1:NP], nullk_d[l])
                for (qo, qw) in QTS:
                    sq = scrp.tile([64, QT], fp16, tag="ksq")
                    nc.scalar.activation(sq[:], ks[:, qo:qo + qw], AF.Square)
                    ssq = ps.tile([1, QT], f32, tag="mm")
                    nc.tensor.matmul(ssq[:], ones64c_b[:], sq[:], start=True, stop=True)
                    sh = vecp.tile([1, QT], f32, tag="sh")
                    nc.scalar.activation(sh[:], ssq[:], AF.Sqrt, bias=eps12[0:1, :])
                    rh = vecp.tile([1, QT], R, tag="rh")
                    nc.vector.reciprocal(rh[:], sh[:])
                    bck = psbc.tile([64, QT], f32, tag="bc")
                    nc.tensor.matmul(bck[:], ones1x64_t[:], rh[:], start=True, stop=True)
                    nc.vector.tensor_mul(kdup[0:64, qo:qo + qw], ks[:, qo:qo + qw], bck[:])
                nc.vector.tensor_copy(kdup[64:128, :], kdup[0:64, :])

                # ---- V (token-major) + ones column ----
                bvr = scrp.tile([1, DH], fp16, tag="bvr")
                nc.sync.dma_start(bvr[:], bv_d[l])
                for t, (to, tw) in enumerate(TCH):
                    pv = ps.tile([128, DH], f32, tag="mm")
                    for c in range(8):
                        nc.tensor.matmul(pv[0:tw, :], xn[:, c * NP + to:c * NP + to + tw],
                                         sw_t[:, 512 + c * 64:512 + (c + 1) * 64],
                                         start=(c == 0), stop=False)
                    nc.tensor.matmul(pv[0:tw, :], ones1xNP_b[:, 0:tw], bvr[:],
                                     start=False, stop=True)
                    nc.scalar.copy(vaug[0:tw, t * 65:t * 65 + 64], pv[0:tw, :])
                nc.sync.dma_start(vaug[7:8, 4 * 65:4 * 65 + 64], nullv_d[l])

                wo_t = load_wo(l)

                # ---- attention + Wo + out-LN, pipelined per query-half ----
                o2 = xn
                for (qo, qw) in QTS:
                    for h in range(H):
                        t, hb = h // 2, (h % 2) * 64
                        av = psacc.tile([65, QT], f32, tag="acc")
                        for c, (ko, kw) in enumerate(KCH):
                            sp = ps.tile([128, QT], f32, tag="mm")
                            nc.tensor.matmul(sp[0:kw, :], kdup[hb:hb + 64, ko:ko + kw],
                                             qhat[hb:hb + 64, t * NP + qo:t * NP + qo + qw],
                                             start=True, stop=True)
                            au0 = attp.tile([128, QT], bf16, tag="au0")
                            nc.scalar.activation(au0[0:kw, :], sp[0:kw, :], AF.Exp)
                            au = attp.tile([128, QT], bf16, tag="au")
                            nc.vector.tensor_mul(
                                au[0:kw, :], au0[0:kw, :],
                                ebias_t[0:kw, (c * 8 + h) * NP + qo:(c * 8 + h) * NP + qo + qw])
                            nc.tensor.matmul(av[:], vaug[0:kw, c * 65:(c + 1) * 65],
                                             au[0:kw, :], start=(c == 0), stop=(c == 4))
                        rd = vecp.tile([1, QT], R, tag="rd")
                        nc.vector.reciprocal(rd[:], av[64:65, :])
                        bco = psbc.tile([64, QT], f32, tag="bc")
                        nc.tensor.matmul(bco[:], ones1x64_t[:], rd[:], start=True, stop=True)
                        bcos = attp.tile([64, QT], fp16, tag="bcos")
                        nc.scalar.copy(bcos[:], bco[:])
                        nc.vector.tensor_mul(oT[hb:hb + 64, t * NP + qo:t * NP + qo + qw],
                                             av[0:64, :], bcos[:])

                    # Wo for this query-half
                    for mt in range(NMT):
                        pl = ps.tile([128, QT], f32, tag="mm")
                        for cc in range(4):
                            nc.tensor.matmul(pl[:], wo_t[:, cc * 1024 + mt * 128:
                                                         cc * 1024 + (mt + 1) * 128],
                                             oT[:, cc * NP + qo:cc * NP + qo + qw],
                                             start=(cc == 0), stop=(cc == 3))
                        nc.scalar.activation(o2[:, mt * NP + qo:mt * NP + qo + qw],
                                             pl[:], AF.Copy)

                    # out-LN + residual for this query-half
                    m_v, r_v = ln_stats(o2, qo, qw, fp16)
                    mb = psbc.tile([128, QT], f32, tag="bc")
                    nc.tensor.matmul(mb[:], ones1x128_t[:], m_v[:],
                                     start=True, stop=True)
                    o2_v = o2[:].rearrange("p (c n) -> p c n", c=NMT)[:, :, qo:qo + qw]
                    nc.vector.tensor_sub(o2_v, o2_v,
                                         mb[:, None, :].broadcast_to([128, NMT, qw]))
                    for mt in range(NMT):
                        rbg = psbc.tile([128, QT], f32, tag="bc")
                        nc.tensor.matmul(rbg[:], og_t[:, mt * 128:(mt + 1) * 128],
                                         r_v[:], start=True, stop=True)
                        seg = o2[:, mt * NP + qo:mt * NP + qo + qw]
                        nc.vector.tensor_mul(seg, seg, rbg[:])
                    xt_v = xt[:].rearrange("p (c n) -> p c n", c=NMT)[:, :, qo:qo + qw]
                    nc.vector.tensor_add(xt_v, xt_v, o2_v)

                ln_to_xn(xt, R)

                # ---- FFN ----
                for q in range(4):
                    ffn_up_quarter(l, q)
                    ffn_down_quarter(l, q)

                if l + 1 < L:
                    wq_t = load_wq(l + 1)
                    sw_t = load_smallw(l + 1)
                    og_t = load_og(l + 1)

            # ================= final stable LN + Wproj =================
            wpr0 = wf1p.tile([128, 4096], fp16, tag="wff1")
            nc.sync.dma_start(wpr0[:], wproj_d[:, 0:4096])
            wpr1 = wf1p.tile([128, 4096], fp16, tag="wff1")
            nc.sync.dma_start(wpr1[:], wproj_d[:, 4096:8192])
            wpr = [wpr0, wpr1]

            xm = finp.tile([128, NP], f32, tag="xm")
            nc.vector.tensor_max(xm[:], xt[:, 0:NP], xt[:, NP:2 * NP])
            for mt in range(2, NMT):
                nc.vector.tensor_max(xm[:], xm[:], xt[:, mt * NP:(mt + 1) * NP])
            mxb = finp.tile([128, NP], f32, tag="mxb")
            nc.gpsimd.partition_all_reduce(mxb[:], xm[:], 128, bass_isa.ReduceOp.max)

            for (qo, qw) in QTS:
                s1p = ps.tile([1, QT], f32, tag="mm")
                s2p = ps.tile([1, QT], f32, tag="mm")
                for mt in range(NMT):
                    seg = xt[:, mt * NP + qo:mt * NP + qo + qw]
                    sq = scrp.tile([128, QT], R, tag="sq")
                    nc.scalar.activation(sq[:], seg, AF.Square)
                    nc.tensor.matmul(s1p[:], ones128c_f[:], seg,
                                     start=(mt == 0), stop=(mt == NMT - 1))
                    nc.tensor.matmul(s2p[:], ones128c_f[:], sq[:],
                                     start=(mt == 0), stop=(mt == NMT - 1))
                m_v = vecp.tile([1, QT], R, tag="m")
                nc.scalar.activation(m_v[:], s1p[:], AF.Copy, scale=1.0 / D)
                q2_v = vecp.tile([1, QT], f32, tag="q2")
                nc.scalar.activation(q2_v[:], s2p[:], AF.Copy, scale=1.0 / D)
                msq_v = vecp.tile([1, QT], f32, tag="msq")
                nc.scalar.activation(msq_v[:], m_v[:], AF.Square)
                v_v = vecp.tile([1, QT], f32, tag="v")
                nc.vector.tensor_sub(v_v[:], q2_v[:], msq_v[:])
                # stable LN: x/M then mean/var scale by M, M^2; fold instead:
                # r = 1/sqrt(var + eps*M^2), xn = (x - m) * r  (M = rowmax)
                mxsq_v = vecp.tile([1, QT], f32, tag="mxsq")
                nc.scalar.activation(mxsq_v[:], mxb[0:1, qo:qo + qw], AF.Square)
                veps_v = vecp.tile([1, QT], f32, tag="veps")
                nc.vector.scalar_tensor_tensor(veps_v[:], mxsq_v[:], EPS, v_v[:],
                                               ALU.mult, ALU.add)
                s_v = vecp.tile([1, QT], f32, tag="s")
                nc.scalar.activation(s_v[:], veps_v[:], AF.Sqrt)
                r_v = vecp.tile([1, QT], R, tag="r")
                nc.vector.reciprocal(r_v[:], s_v[:])
                mr_v = vecp.tile([1, QT], R, tag="mr")
                nc.vector.tensor_mul(mr_v[:], m_v[:], r_v[:])
                rb = psbc.tile([128, QT], f32, tag="bc")
                nc.tensor.matmul(rb[:], ones1x128_t[:], r_v[:], start=True, stop=True)
                mrb = psbc.tile([128, QT], f32, tag="bc")
                nc.tensor.matmul(mrb[:], ones1x128_t[:], mr_v[:], start=True, stop=True)
                for mt in range(NMT):
                    t1 = scrp.tile([128, QT], f32, tag="t1")
                    nc.vector.tensor_mul(t1[:], xt[:, mt * NP + qo:mt * NP + qo + qw], rb[:])
                    nc.vector.tensor_sub(xn[:, mt * NP + qo:mt * NP + qo + qw],
                                         t1[:], mrb[:])

            for t, (to, tw) in enumerate(TCH):
                rtw = min(tw, max(0, N - to))
                if rtw == 0:
                    continue
                for half in range(2):
                    pn = psacc.tile([128, 512], f32, tag="acc")
                    for c in range(8):
                        nc.tensor.matmul(pn[0:tw, :],
                                         xn[:, c * NP + to:c * NP + to + tw],
                                         wpr[half][:, c * 512:(c + 1) * 512],
                                         start=(c == 0), stop=(c == 7))
                    st = finp.tile([128, 512], f32, tag=f"outst{half}")
                    nc.vector.tensor_copy(st[0:rtw, :], pn[0:rtw, :])
                    nc.sync.dma_start(out_d[to:to + rtw, half * 512:(half + 1) * 512],
                                      st[0:rtw, :])

    nc.compile()
    return nc


_CACHE = {}


def _get_program():
    if 'nc' not in _CACHE:
        _CACHE['nc'] = _build()
    return _CACHE['nc']


def kernel(**inputs) -> np.ndarray:
    from concourse.bass_utils import run_bass_kernel_spmd
    host = _host_prep(inputs)
    nc = _get_program()
    shared = {k: v for k, v in host.items() if k != 'xT'}
    in_maps = [dict(shared, xT=np.ascontiguousarray(host['xT'][b])) for b in range(B)]
    res = run_bass_kernel_spmd(nc, in_maps, list(range(B)))
    out = np.stack([res.results[b]['out'] for b in range(B)], axis=0)
    _CACHE['last_results'] = res
    return out
